# revision 12
# baseline (speedup 1.0000x reference)
"""Cosine multi-head attention (h=1) Trainium2 kernel.

Math (reference):
    context = query @ Wq.T + bq                  [B, S, HD]
    ctx     = context * weight_tensor[0]         (elementwise over HD)
    ctx_n   = ctx / max(||ctx||_2, eps)          (normalize over HD)
    scores  = ctx_n @ ctx_n.T                    [B, S, S]
    out     = softmax(scores, axis=-1)

Device strategy (8 cores, SPMD):
    core c handles batch b = c//2, row-half h = c%2.  The host rotates the
    batch's rows so each core's own 2048 rows come first, transposes to
    qT [D, S], casts to fp16, and folds weight_tensor into Wq:
    M = 256 * diag(w) @ Wq (fp16; the 256 cancels in the normalize and
    keeps M's entries out of fp16-subnormal range), c0 = 256 * w * bq.

    Everything runs single-pass fp16 on the PE (the harness tolerance is
    2e-2; measured end-to-end error of this scheme is ~1e-4).  The
    normalization uses ACT ln->exp (inv = exp(-0.5*ln(n2+eps^2))) because
    Ln and Exp share one activation table set with the softmax Exp, so the
    scalar engine never reloads tables (a reciprocal_sqrt would force
    ~2.6us table thrash per switch in the streamed schedule).

    Streaming: q arrives in 8 column groups of 512; each group's context
    matmul, norm chain and normalized fp16 cast complete independently, so
    the gram/softmax for row-chunk i (128 rows) starts as soon as the
    first 4 groups (2048 cols, which are also all 16 lhsT chunks) are
    ready -- the scalar engine's EXP stream (the 63us roofline of this
    kernel: 8.4M softmax elements at 1 col/cycle) starts ~12us into the
    kernel and runs back-to-back.

    Softmax needs no max-subtraction (cosine scores are in [-1,1]); the
    Exp is biased by -6*ln2 so the fp16 numerator tile stays in normal
    range, and the final (e * rec) * 64 fused DVE rescale hands the host
    values 64x the true softmax, which the gather divides back out.

    Output is fp16 (error ~5e-4 of max, well inside tolerance), halving
    the dominant output DMA traffic.  The h=1 cores' columns are rotated
    by 2048; the host gather undoes it.
"""

import numpy as np
from contextlib import ExitStack

B, S, D, HD = 4, 4096, 1024, 120
ROWS = S // 2          # rows of the score matrix each core produces
G = 8                  # column groups of 512
GW = S // G            # 512
DC = D // 128          # 8 contraction chunks
NCHUNK = ROWS // 128   # 16 row chunks
EPS2 = 1e-24           # matches the reference's F.normalize eps of 1e-12
LN2 = 0.6931471805599453
EBIAS = -6.0 * LN2     # exp(score + EBIAS) = exp(score)/64, fp16-normal
N_CORES = 8

_NC_CACHE = {}


def _patch_act_tables(bacc, mybir):
    """Force Ln/Exp to resolve to the one table set containing both
    (natural_log_exp_and_others), so the streamed ln->exp norm chain and
    the softmax Exp share a single ACT_TABLE_LOAD instead of thrashing
    (~1.3us per reload).  Set order/indices are preserved; only the
    redundant Exp/Ln entries of the other sets are hidden from the
    placement pass.  Returns a restore handle."""
    AF = mybir.ActivationFunctionType
    orig = bacc.get_activation_tables

    def patched(arch):
        tables = orig(arch)
        both = [n for n, fns in tables.items()
                if AF.Exp in fns and AF.Ln in fns]
        if both:
            keep = both[0]
            for n, fns in tables.items():
                if n != keep:
                    fns.discard(AF.Exp)
                    fns.discard(AF.Ln)
        return tables

    bacc.get_activation_tables = patched
    return orig


def _build_nc():
    import concourse.bacc as bacc
    import concourse.tile as tile
    from concourse import mybir

    f32 = mybir.dt.float32
    f16 = mybir.dt.float16
    bf16 = mybir.dt.bfloat16
    AF = mybir.ActivationFunctionType
    ALU = mybir.AluOpType
    _orig_tables = _patch_act_tables(bacc, mybir)
    nc = bacc.Bacc("TRN2", target_bir_lowering=False, debug=False,
                   num_devices=N_CORES)

    q_p = nc.declare_dram_parameter("q_p", [G, 128, DC, GW], bf16,
                                    isOutput=False)
    mt_p = nc.declare_dram_parameter("mt_p", [128, DC, HD], bf16,
                                     isOutput=False)
    c0_p = nc.declare_dram_parameter("c0_p", [HD, 1], f32, isOutput=False)
    out = nc.declare_dram_parameter("out", [ROWS, S], f16, isOutput=True)

    with ExitStack() as ctx:
        tc = ctx.enter_context(tile.TileContext(nc))
        singles = ctx.enter_context(tc.tile_pool(name="singles", bufs=1))
        qpool = ctx.enter_context(tc.tile_pool(name="qpool", bufs=8))
        ctpool = ctx.enter_context(tc.tile_pool(name="ctpool", bufs=3))
        sqpool = ctx.enter_context(tc.tile_pool(name="sqpool", bufs=2))
        invpool = ctx.enter_context(tc.tile_pool(name="invpool", bufs=2))
        epool = ctx.enter_context(tc.tile_pool(name="epool", bufs=7))
        spool = ctx.enter_context(tc.tile_pool(name="spool", bufs=7))
        ps = ctx.enter_context(tc.tile_pool(name="ps", bufs=2, space="PSUM"))

        # constants first in the DMA queues (tiny)
        mt_sb = singles.tile([128, DC, HD], bf16, tag="mt")
        nc.sync.dma_start(out=mt_sb[:], in_=mt_p[:])
        c0_sb = singles.tile([HD, 1], f32, tag="c0")
        nc.sync.dma_start(out=c0_sb[:], in_=c0_p[:])
        ones_sq = singles.tile([HD, HD], bf16, tag="ones")
        nc.vector.memset(ones_sq[:], 1.0)
        # bias constants for the ACT ops (floats other than 0/1 need APs)
        eps2_sb = singles.tile([HD, 1], f32, tag="eps2")
        nc.vector.memset(eps2_sb[:], EPS2)
        ebias_sb = singles.tile([128, 1], f32, tag="ebias")
        nc.vector.memset(ebias_sb[:], EBIAS)

        # all of q streams from t0 in 1MB group DMAs (128 x 8KB descriptors)
        q_sb = []
        for g in range(G):
            qt = qpool.tile([128, DC, GW], bf16, tag="q", name=f"q{g}")
            nparts = 4 if g < 2 else 2
            h = DC // nparts
            for p in range(nparts):
                nc.sync.dma_start(out=qt[:, p * h:(p + 1) * h, :],
                                  in_=q_p[g, :, p * h:(p + 1) * h, :])
            q_sb.append(qt)

        # normalized context, fp16, resident for the whole gram phase
        cn = singles.tile([HD, G, GW], bf16, tag="cn")

        ct_sbs = [None] * G
        invs = [None] * G

        def p1_mm(g):
            """context matmul for column group g."""
            ct_ps = ps.tile([HD, GW], f32, tag="ps", name=f"ct_ps{g}")
            for c in range(DC):
                nc.tensor.matmul(ct_ps[:], lhsT=mt_sb[:, c, :],
                                 rhs=q_sb[g][:, c, :],
                                 start=(c == 0), stop=(c == DC - 1))
            ct_sb = ctpool.tile([HD, GW], f32, tag="ct", name=f"ct{g}")
            # bias + PSUM->SBUF move in one DVE op (frees the PSUM slot)
            nc.vector.tensor_scalar_add(ct_sb[:], ct_ps[:], c0_sb[:])
            ctsq = sqpool.tile([HD, GW], bf16, tag="sq", name=f"sq{g}")
            nc.vector.tensor_mul(ctsq[:], ct_sb[:], ct_sb[:])
            ct_sbs[g] = (ct_sb, ctsq)

        def p1_n2(g):
            """norm reduction + inv chain for group g."""
            ct_sb, ctsq = ct_sbs[g]
            n2_ps = ps.tile([HD, GW], f32, tag="ps", name=f"n2_ps{g}")
            nc.tensor.matmul(n2_ps[:], lhsT=ones_sq[:], rhs=ctsq[:],
                             start=True, stop=True)
            # inv = (n2 + eps^2) ** -0.5 via ln->exp (same ACT table set as
            # the softmax Exp -> no table reloads anywhere in the kernel)
            nc.scalar.activation(out=n2_ps[:], in_=n2_ps[:], func=AF.Ln,
                                 bias=eps2_sb[:])
            inv = invpool.tile([HD, GW], f32, tag="inv", name=f"inv{g}")
            nc.scalar.activation(out=inv[:], in_=n2_ps[:], func=AF.Exp,
                                 scale=-0.5)
            invs[g] = inv

        def p1_cn(g):
            nc.vector.tensor_mul(cn[:, g, :], ct_sbs[g][0][:], invs[g][:])

        e2 = [None] * NCHUNK
        sums = [None] * NCHUNK

        def p3(i, jg, split=False, only_h=None):
            """gram + exp for row chunk i, column half jg (2048 cols).
            split=True runs the half as two 1024-col sub-tiles (only_h
            emits just one of them) so chunk 0's exp can start as soon as
            the first two column groups' chains finish."""
            if jg == 0 and only_h in (None, 0):
                e2[i] = epool.tile([128, S], f16, tag="e2", name=f"e{i}")
                sums[i] = spool.tile([128, 4], f32, tag="sums",
                                     name=f"sums{i}")
            lhsT = cn[:, i // 4, (i % 4) * 128:(i % 4) * 128 + 128]
            nsub = 2 if split else 1
            w = 2048 // nsub
            hs = range(nsub) if only_h is None else [only_h]
            for h in hs:
                r_ps = ps.tile([128, w], f32, tag="ps", name=f"r{i}_{jg}_{h}")
                for k in range(w // GW):
                    g = jg * 4 + h * (w // GW) + k
                    nc.tensor.matmul(r_ps[:, k * GW:(k + 1) * GW],
                                     lhsT=lhsT, rhs=cn[:, g, :],
                                     start=True, stop=True)
                col = jg * 2048 + h * w
                nc.scalar.activation(
                    out=e2[i][:, col:col + w], in_=r_ps[:],
                    func=AF.Exp, bias=ebias_sb[:],
                    accum_out=sums[i][:, 2 * jg + h:2 * jg + h + 1])
            if jg == 1 and (only_h is None or only_h == nsub - 1):
                tot = spool.tile([128, 1], f32, tag="tot", name=f"tot{i}")
                if split:
                    nc.vector.reduce_sum(tot[:], sums[i][:],
                                         axis=mybir.AxisListType.X)
                else:
                    nc.vector.tensor_add(tot[:], sums[i][:, 0:1],
                                         sums[i][:, 2:3])
                rec = spool.tile([128, 1], f32, tag="rec", name=f"rec{i}")
                nc.vector.reciprocal(rec[:], tot[:])
                # row-normalize and undo the 2^-6 exp bias in one pass;
                # host divides the final fp32 result by 64.  The final
                # chunk goes in halves so rescale and out-DMA overlap.
                nparts = 8 if i == NCHUNK - 1 else 1
                half = S // nparts
                for h in range(nparts):
                    sl = slice(h * half, (h + 1) * half)
                    nc.vector.tensor_scalar(out=e2[i][:, sl],
                                            in0=e2[i][:, sl],
                                            scalar1=rec[:], scalar2=64.0,
                                            op0=ALU.mult, op1=ALU.mult)
                    nc.sync.dma_start(
                        out=out[i * 128:(i + 1) * 128, sl],
                        in_=e2[i][:, sl])

        # schedule: pipeline the 8 group chains at DMA rate (the mm of
        # group g+1 is emitted before the chain of group g so the in-order
        # PE/DVE queues never stall on the cross-engine chain), and
        # interleave early row-chunk jg0 work with the tail group chains
        # so the scalar engine's exp stream starts ~20us in and runs
        # gap-free.  Each p3 emission strictly follows the p1_cn of every
        # group it reads (in-order engines race on anything emitted
        # reader-before-writer).
        p1_mm(0)
        p1_mm(1)
        p1_n2(0)
        p1_mm(2)
        p1_n2(1)
        p1_cn(0)
        p1_cn(1)
        p3(0, 0, split=True, only_h=0)
        p1_mm(3)
        p1_n2(2)
        p1_cn(2)
        p1_mm(4)
        p1_n2(3)
        p1_cn(3)
        p3(0, 0, split=True, only_h=1)
        p1_mm(5)
        p1_n2(4)
        p1_cn(4)
        p3(1, 0)
        p1_mm(6)
        p1_n2(5)
        p1_cn(5)
        p3(2, 0)
        p1_mm(7)
        p1_n2(6)
        p1_cn(6)
        p3(3, 0)
        p1_n2(7)
        p1_cn(7)
        p3(4, 0)
        p3(5, 0)
        p3(0, 1, split=True)
        for i in range(1, 6):
            p3(i, 1)
        for i in range(6, NCHUNK):
            p3(i, 0)
            p3(i, 1)

    try:
        nc.compile()
    finally:
        bacc.get_activation_tables = _orig_tables
    return nc


def _get_nc():
    if "nc" not in _NC_CACHE:
        _NC_CACHE["nc"] = _build_nc()
    return _NC_CACHE["nc"]


def _make_in_maps(inputs):
    query = np.asarray(inputs["query"], dtype=np.float32)
    Wq = np.asarray(inputs["Wq"], dtype=np.float32)
    bq = np.asarray(inputs["bq"], dtype=np.float32)
    w = np.asarray(inputs["weight_tensor"], dtype=np.float32)

    w0 = w.reshape(-1)[:HD]
    m = (w0[:, None] * Wq).T                               # [D, HD]
    # [D, HD] -> [128(p), DC(c), HD], D-index = c*128 + p
    import ml_dtypes
    bf = ml_dtypes.bfloat16
    mt_np = np.ascontiguousarray(
        m.reshape(DC, 128, HD).transpose(1, 0, 2)).astype(bf)
    c0_np = np.ascontiguousarray((w0 * bq)[:, None]).astype(np.float32)

    in_maps = []
    for c in range(N_CORES):
        b, h = c // 2, c % 2
        qb = query[b]
        if h:
            qb = np.concatenate([qb[ROWS:], qb[:ROWS]], axis=0)
        qT = qb.T.astype(bf)                               # [D, S]
        # [D, S] -> [G(g), 128(p), DC(c), GW(j)]: row = c*128+p, col = g*512+j
        q_np = np.ascontiguousarray(
            qT.reshape(DC, 128, G, GW).transpose(2, 1, 0, 3))
        in_maps.append({"q_p": q_np, "mt_p": mt_np, "c0_p": c0_np})
    return in_maps


def _gather(results):
    full = np.empty((B, S, S), dtype=np.float32)
    inv64 = np.float32(1.0 / 64.0)
    for c in range(N_CORES):
        b, h = c // 2, c % 2
        r = results[c]["out"].astype(np.float32) * inv64
        if h == 0:
            full[b, :ROWS] = r
        else:
            full[b, ROWS:, ROWS:] = r[:, :ROWS]
            full[b, ROWS:, :ROWS] = r[:, ROWS:]
    return full


def kernel(**inputs):
    from concourse.bass_utils import run_bass_kernel_spmd

    in_maps = _make_in_maps(inputs)
    nc = _get_nc()
    res = run_bass_kernel_spmd(nc, in_maps, list(range(N_CORES))).results
    return _gather(res)


def _register_ntff_hook():
    """Register the axon NTFF profile hook that the agent image's antenv
    package lacks (see trn_boot.py) so trace=True yields exec_time_ns."""
    import sys
    import types
    try:
        import antenv.axon_hooks  # noqa: F401
        return True
    except ImportError:
        pass
    try:
        from trn_agent_boot.trn_boot import _ntff_profile_via_ctypes
        hook = _ntff_profile_via_ctypes("/opt/axon/libaxon_pjrt.so")
    except Exception:
        return False
    if hook is None:
        return False
    mod = types.ModuleType("antenv.axon_hooks")
    mod._hook = hook
    mod.get_axon_ntff_profile_hook = lambda: mod._hook
    mod.set_axon_ntff_profile_hook = lambda h: setattr(mod, "_hook", h)
    sys.modules["antenv.axon_hooks"] = mod
    import antenv
    antenv.axon_hooks = mod
    return True


def profile_once(inputs, trace_cores=None):
    """Re-run the kernel with NTFF profiling; returns max exec_time_ns."""
    import tempfile
    import concourse.bass_utils as bu

    _register_ntff_hook()
    # avoid the cloud artifact upload inside the trace path
    bu.upload_artifacts = lambda tmpdir: tmpdir

    in_maps = _make_in_maps(inputs)
    nc = _get_nc()
    tmpdir = tempfile.mkdtemp(prefix="ntff_")
    r = bu.run_bass_kernel_spmd(nc, in_maps, list(range(N_CORES)),
                                trace=True, trace_cores=trace_cores,
                                tmpdir=tmpdir)
    print(f"trace dir: {tmpdir}")
    if r.exec_time_ns is not None:
        print(f"mean exec: {r.mean_exec_time_ns} ns, "
              f"max core: {r.max_exec_time_core_id}")
    return r.exec_time_ns


# revision 13
# speedup vs baseline: 1.0181x; 1.0181x over previous
"""Cosine multi-head attention (h=1) Trainium2 kernel.

Math (reference):
    context = query @ Wq.T + bq                  [B, S, HD]
    ctx     = context * weight_tensor[0]         (elementwise over HD)
    ctx_n   = ctx / max(||ctx||_2, eps)          (normalize over HD)
    scores  = ctx_n @ ctx_n.T                    [B, S, S]
    out     = softmax(scores, axis=-1)

Device strategy (8 cores, SPMD):
    core c handles batch b = c//2, row-half h = c%2.  The host rotates
    the batch's rows so each core's own 2048 rows come first, transposes
    to qT [D, S], casts to bf16, and folds weight_tensor into Wq:
    M = diag(w) @ Wq (bf16), c0 = w * bq.

    All matmuls are single-pass bf16 (the harness tolerance is 2e-2;
    measured end-to-end error is ~3.4e-3).  fp16 operands would halve
    the rounding error but run 2-pass on the PE (~2x slower); the
    3-pass compensated-bf16 scheme of the first version of this kernel
    cost ~3x PE time for accuracy far beyond what is needed.

    The normalization inv = exp(-0.5*ln(n2+eps^2)) runs on the scalar
    engine because Ln and Exp share one activation table set
    (natural_log_exp_and_others) with the softmax Exp -- a single
    ACT_TABLE_LOAD for the whole kernel.  _patch_act_tables forces the
    placement pass to pick that set (it otherwise alternates the
    ln-only/exp-only sets, ~1.3us per reload, 16 reloads).

    Streaming: q arrives in 8 column groups of 512 at full DMA rate
    (all group tiles are resident so transfers never stall on pool
    slots); each group's context matmul, ln->exp norm chain, and bf16
    normalized cast complete independently.  Emission interleaves the
    in-order engine queues so nothing blocks: group g+1's matmuls are
    emitted before group g's norm chain, and the early row chunks' gram
    halves (which need only the first 4 groups) fill the scalar engine
    while the last groups stream in.  Every emission strictly follows
    the writers of everything it reads -- the dependency tracker cannot
    see writes that have not been emitted yet, so a reader emitted
    early races on stale SBUF.

    The softmax EXP stream is the roofline of this kernel: 8.4M
    elements/core at 1 column/cycle on the scalar engine (~60us), plus
    the norm chains, accumulator reads and one table load (~78us busy).
    It starts ~18us in (input DMA + first chains) and runs with ~9us of
    total gaps.  PSUM is the tight resource: 2 slots of 4 banks ping-
    pong between the phase-1 context/norm tiles and the [128,2048] gram
    tiles; the emission order above is also what keeps the slot ring
    deadlock- and stall-free.

    Softmax needs no max-subtraction (cosine scores are in [-1,1]); the
    Exp is biased by -6*ln2 so the fp16 numerator tile stays in normal
    range (raw values down to 3e-5 would hit fp16 subnormals), and the
    fused (e * rec) * 64 DVE rescale hands the host values 64x the true
    softmax, which the gather divides back out exactly.

    Output is fp16 (error ~5e-4 of max, well inside tolerance), halving
    the dominant output DMA traffic.  Row sums come free from the EXP's
    accum_out.  The h=1 cores' columns are rotated by 2048; the host
    gather undoes it.

    Not taken: exploiting the symmetry of the diagonal 2048x2048 block
    (exp only the upper triangle, mirror via SBUF->SBUF transpose-DMA)
    saved ~14us of scalar-engine work on paper, but the XBAR transpose
    path raced nondeterministically with the compute pipeline (NaNs)
    and its per-DMA overhead erased the gain; fp8 I/O fails the 2e-2
    gate; GPSIMD/DVE cannot run exp at a competitive rate.
"""
import numpy as np
from contextlib import ExitStack

B, S, D, HD = 4, 4096, 1024, 120
ROWS = S // 2          # rows of the score matrix each core produces
G = 8                  # column groups of 512
GW = S // G            # 512
DC = D // 128          # 8 contraction chunks
NCHUNK = ROWS // 128   # 16 row chunks
EPS2 = 1e-24           # matches the reference's F.normalize eps of 1e-12
LN2 = 0.6931471805599453
EBIAS = -6.0 * LN2     # exp(score + EBIAS) = exp(score)/64, fp16-normal
N_CORES = 8

_NC_CACHE = {}


def _patch_act_tables(bacc, mybir):
    """Force Ln/Exp to resolve to the one table set containing both
    (natural_log_exp_and_others), so the streamed ln->exp norm chain and
    the softmax Exp share a single ACT_TABLE_LOAD instead of thrashing
    (~1.3us per reload).  Set order/indices are preserved; only the
    redundant Exp/Ln entries of the other sets are hidden from the
    placement pass.  Returns a restore handle."""
    AF = mybir.ActivationFunctionType
    orig = bacc.get_activation_tables

    def patched(arch):
        tables = orig(arch)
        both = [n for n, fns in tables.items()
                if AF.Exp in fns and AF.Ln in fns]
        if both:
            keep = both[0]
            for n, fns in tables.items():
                if n != keep:
                    fns.discard(AF.Exp)
                    fns.discard(AF.Ln)
        return tables

    bacc.get_activation_tables = patched
    return orig


def _build_nc():
    import concourse.bacc as bacc
    import concourse.tile as tile
    from concourse import mybir

    f32 = mybir.dt.float32
    f16 = mybir.dt.float16
    bf16 = mybir.dt.bfloat16
    AF = mybir.ActivationFunctionType
    ALU = mybir.AluOpType
    _orig_tables = _patch_act_tables(bacc, mybir)
    nc = bacc.Bacc("TRN2", target_bir_lowering=False, debug=False,
                   num_devices=N_CORES)

    q_p = nc.declare_dram_parameter("q_p", [G, 128, DC, GW], bf16,
                                    isOutput=False)
    mt_p = nc.declare_dram_parameter("mt_p", [128, DC, HD], bf16,
                                     isOutput=False)
    c0_p = nc.declare_dram_parameter("c0_p", [HD, 1], f32, isOutput=False)
    out = nc.declare_dram_parameter("out", [ROWS, S], f16, isOutput=True)

    with ExitStack() as ctx:
        tc = ctx.enter_context(tile.TileContext(nc))
        singles = ctx.enter_context(tc.tile_pool(name="singles", bufs=1))
        qpool = ctx.enter_context(tc.tile_pool(name="qpool", bufs=8))
        ctpool = ctx.enter_context(tc.tile_pool(name="ctpool", bufs=3))
        sqpool = ctx.enter_context(tc.tile_pool(name="sqpool", bufs=2))
        invpool = ctx.enter_context(tc.tile_pool(name="invpool", bufs=2))
        epool = ctx.enter_context(tc.tile_pool(name="epool", bufs=7))
        spool = ctx.enter_context(tc.tile_pool(name="spool", bufs=7))
        ps = ctx.enter_context(tc.tile_pool(name="ps", bufs=2, space="PSUM"))

        # constants first in the DMA queues (tiny)
        mt_sb = singles.tile([128, DC, HD], bf16, tag="mt")
        nc.sync.dma_start(out=mt_sb[:], in_=mt_p[:])
        c0_sb = singles.tile([HD, 1], f32, tag="c0")
        nc.sync.dma_start(out=c0_sb[:], in_=c0_p[:])
        ones_sq = singles.tile([HD, HD], bf16, tag="ones")
        nc.vector.memset(ones_sq[:], 1.0)
        # bias constants for the ACT ops (floats other than 0/1 need APs)
        eps2_sb = singles.tile([HD, 1], f32, tag="eps2")
        nc.vector.memset(eps2_sb[:], EPS2)
        ebias_sb = singles.tile([128, 1], f32, tag="ebias")
        nc.vector.memset(ebias_sb[:], EBIAS)

        # all of q streams from t0 in 1MB group DMAs (128 x 8KB descriptors)
        q_sb = []
        for g in range(G):
            qt = qpool.tile([128, DC, GW], bf16, tag="q", name=f"q{g}")
            h = DC // 2
            nc.sync.dma_start(out=qt[:, :h, :], in_=q_p[g, :, :h, :])
            nc.sync.dma_start(out=qt[:, h:, :], in_=q_p[g, :, h:, :])
            q_sb.append(qt)

        # normalized context, fp16, resident for the whole gram phase
        cn = singles.tile([HD, G, GW], bf16, tag="cn")

        ct_sbs = [None] * G
        invs = [None] * G

        def p1_mm(g):
            """context matmul for column group g."""
            ct_ps = ps.tile([HD, GW], f32, tag="ps", name=f"ct_ps{g}")
            for c in range(DC):
                nc.tensor.matmul(ct_ps[:], lhsT=mt_sb[:, c, :],
                                 rhs=q_sb[g][:, c, :],
                                 start=(c == 0), stop=(c == DC - 1))
            ct_sb = ctpool.tile([HD, GW], f32, tag="ct", name=f"ct{g}")
            # bias + PSUM->SBUF move in one DVE op (frees the PSUM slot)
            nc.vector.tensor_scalar_add(ct_sb[:], ct_ps[:], c0_sb[:])
            ctsq = sqpool.tile([HD, GW], bf16, tag="sq", name=f"sq{g}")
            nc.vector.tensor_mul(ctsq[:], ct_sb[:], ct_sb[:])
            ct_sbs[g] = (ct_sb, ctsq)

        def p1_n2(g):
            """norm reduction + inv chain for group g."""
            ct_sb, ctsq = ct_sbs[g]
            n2_ps = ps.tile([HD, GW], f32, tag="ps", name=f"n2_ps{g}")
            nc.tensor.matmul(n2_ps[:], lhsT=ones_sq[:], rhs=ctsq[:],
                             start=True, stop=True)
            # inv = (n2 + eps^2) ** -0.5 via ln->exp (same ACT table set as
            # the softmax Exp -> no table reloads anywhere in the kernel)
            nc.scalar.activation(out=n2_ps[:], in_=n2_ps[:], func=AF.Ln,
                                 bias=eps2_sb[:])
            inv = invpool.tile([HD, GW], f32, tag="inv", name=f"inv{g}")
            nc.scalar.activation(out=inv[:], in_=n2_ps[:], func=AF.Exp,
                                 scale=-0.5)
            invs[g] = inv

        def p1_cn(g):
            nc.vector.tensor_mul(cn[:, g, :], ct_sbs[g][0][:], invs[g][:])

        e2 = [None] * NCHUNK
        sums = [None] * NCHUNK

        def p3(i, jg, split=False, only_h=None):
            """gram + exp for row chunk i, column half jg (2048 cols).
            split=True runs the half as two 1024-col sub-tiles (only_h
            emits just one of them) so chunk 0's exp can start as soon as
            the first two column groups' chains finish."""
            if jg == 0 and only_h in (None, 0):
                e2[i] = epool.tile([128, S], f16, tag="e2", name=f"e{i}")
                sums[i] = spool.tile([128, 4], f32, tag="sums",
                                     name=f"sums{i}")
            lhsT = cn[:, i // 4, (i % 4) * 128:(i % 4) * 128 + 128]
            nsub = 2 if split else 1
            w = 2048 // nsub
            hs = range(nsub) if only_h is None else [only_h]
            for h in hs:
                r_ps = ps.tile([128, w], f32, tag="ps", name=f"r{i}_{jg}_{h}")
                for k in range(w // GW):
                    g = jg * 4 + h * (w // GW) + k
                    nc.tensor.matmul(r_ps[:, k * GW:(k + 1) * GW],
                                     lhsT=lhsT, rhs=cn[:, g, :],
                                     start=True, stop=True)
                col = jg * 2048 + h * w
                nc.scalar.activation(
                    out=e2[i][:, col:col + w], in_=r_ps[:],
                    func=AF.Exp, bias=ebias_sb[:],
                    accum_out=sums[i][:, 2 * jg + h:2 * jg + h + 1])
            if jg == 1 and (only_h is None or only_h == nsub - 1):
                tot = spool.tile([128, 1], f32, tag="tot", name=f"tot{i}")
                if split:
                    nc.vector.reduce_sum(tot[:], sums[i][:],
                                         axis=mybir.AxisListType.X)
                else:
                    nc.vector.tensor_add(tot[:], sums[i][:, 0:1],
                                         sums[i][:, 2:3])
                rec = spool.tile([128, 1], f32, tag="rec", name=f"rec{i}")
                nc.vector.reciprocal(rec[:], tot[:])
                # row-normalize and undo the 2^-6 exp bias in one pass;
                # host divides the final fp32 result by 64.  The final
                # chunk goes in halves so rescale and out-DMA overlap.
                nparts = 4 if i == NCHUNK - 1 else 1
                half = S // nparts
                for h in range(nparts):
                    sl = slice(h * half, (h + 1) * half)
                    nc.vector.tensor_scalar(out=e2[i][:, sl],
                                            in0=e2[i][:, sl],
                                            scalar1=rec[:], scalar2=64.0,
                                            op0=ALU.mult, op1=ALU.mult)
                    nc.sync.dma_start(
                        out=out[i * 128:(i + 1) * 128, sl],
                        in_=e2[i][:, sl])

        # schedule: pipeline the 8 group chains at DMA rate (the mm of
        # group g+1 is emitted before the chain of group g so the in-order
        # PE/DVE queues never stall on the cross-engine chain), and
        # interleave early row-chunk jg0 work with the tail group chains
        # so the scalar engine's exp stream starts ~20us in and runs
        # gap-free.  Each p3 emission strictly follows the p1_cn of every
        # group it reads (in-order engines race on anything emitted
        # reader-before-writer).
        p1_mm(0)
        p1_mm(1)
        p1_n2(0)
        p1_mm(2)
        p1_n2(1)
        p1_cn(0)
        p1_cn(1)
        p3(0, 0, split=True, only_h=0)
        p1_mm(3)
        p1_n2(2)
        p1_cn(2)
        p1_mm(4)
        p1_n2(3)
        p1_cn(3)
        p3(0, 0, split=True, only_h=1)
        p1_mm(5)
        p1_n2(4)
        p1_cn(4)
        p3(1, 0)
        p1_mm(6)
        p1_n2(5)
        p1_cn(5)
        p3(2, 0)
        p1_mm(7)
        p1_n2(6)
        p1_cn(6)
        p3(3, 0)
        p1_n2(7)
        p1_cn(7)
        p3(4, 0)
        p3(5, 0)
        p3(0, 1, split=True)
        for i in range(1, 6):
            p3(i, 1)
        for i in range(6, NCHUNK):
            p3(i, 0)
            p3(i, 1)

    try:
        nc.compile()
    finally:
        bacc.get_activation_tables = _orig_tables
    return nc


def _get_nc():
    if "nc" not in _NC_CACHE:
        _NC_CACHE["nc"] = _build_nc()
    return _NC_CACHE["nc"]


def _make_in_maps(inputs):
    query = np.asarray(inputs["query"], dtype=np.float32)
    Wq = np.asarray(inputs["Wq"], dtype=np.float32)
    bq = np.asarray(inputs["bq"], dtype=np.float32)
    w = np.asarray(inputs["weight_tensor"], dtype=np.float32)

    w0 = w.reshape(-1)[:HD]
    m = (w0[:, None] * Wq).T                               # [D, HD]
    # [D, HD] -> [128(p), DC(c), HD], D-index = c*128 + p
    import ml_dtypes
    bf = ml_dtypes.bfloat16
    mt_np = np.ascontiguousarray(
        m.reshape(DC, 128, HD).transpose(1, 0, 2)).astype(bf)
    c0_np = np.ascontiguousarray((w0 * bq)[:, None]).astype(np.float32)

    in_maps = []
    for c in range(N_CORES):
        b, h = c // 2, c % 2
        qb = query[b]
        if h:
            qb = np.concatenate([qb[ROWS:], qb[:ROWS]], axis=0)
        qT = qb.T.astype(bf)                               # [D, S]
        # [D, S] -> [G(g), 128(p), DC(c), GW(j)]: row = c*128+p, col = g*512+j
        q_np = np.ascontiguousarray(
            qT.reshape(DC, 128, G, GW).transpose(2, 1, 0, 3))
        in_maps.append({"q_p": q_np, "mt_p": mt_np, "c0_p": c0_np})
    return in_maps


def _gather(results):
    full = np.empty((B, S, S), dtype=np.float32)
    inv64 = np.float32(1.0 / 64.0)
    for c in range(N_CORES):
        b, h = c // 2, c % 2
        r = results[c]["out"].astype(np.float32) * inv64
        if h == 0:
            full[b, :ROWS] = r
        else:
            full[b, ROWS:, ROWS:] = r[:, :ROWS]
            full[b, ROWS:, :ROWS] = r[:, ROWS:]
    return full


def kernel(**inputs):
    from concourse.bass_utils import run_bass_kernel_spmd

    in_maps = _make_in_maps(inputs)
    nc = _get_nc()
    res = run_bass_kernel_spmd(nc, in_maps, list(range(N_CORES))).results
    return _gather(res)


def _register_ntff_hook():
    """Register the axon NTFF profile hook that the agent image's antenv
    package lacks (see trn_boot.py) so trace=True yields exec_time_ns."""
    import sys
    import types
    try:
        import antenv.axon_hooks  # noqa: F401
        return True
    except ImportError:
        pass
    try:
        from trn_agent_boot.trn_boot import _ntff_profile_via_ctypes
        hook = _ntff_profile_via_ctypes("/opt/axon/libaxon_pjrt.so")
    except Exception:
        return False
    if hook is None:
        return False
    mod = types.ModuleType("antenv.axon_hooks")
    mod._hook = hook
    mod.get_axon_ntff_profile_hook = lambda: mod._hook
    mod.set_axon_ntff_profile_hook = lambda h: setattr(mod, "_hook", h)
    sys.modules["antenv.axon_hooks"] = mod
    import antenv
    antenv.axon_hooks = mod
    return True


def profile_once(inputs, trace_cores=None):
    """Re-run the kernel with NTFF profiling; returns max exec_time_ns."""
    import tempfile
    import concourse.bass_utils as bu

    _register_ntff_hook()
    # avoid the cloud artifact upload inside the trace path
    bu.upload_artifacts = lambda tmpdir: tmpdir

    in_maps = _make_in_maps(inputs)
    nc = _get_nc()
    tmpdir = tempfile.mkdtemp(prefix="ntff_")
    r = bu.run_bass_kernel_spmd(nc, in_maps, list(range(N_CORES)),
                                trace=True, trace_cores=trace_cores,
                                tmpdir=tmpdir)
    print(f"trace dir: {tmpdir}")
    if r.exec_time_ns is not None:
        print(f"mean exec: {r.mean_exec_time_ns} ns, "
              f"max core: {r.max_exec_time_core_id}")
    return r.exec_time_ns
